# revision 21
# baseline (speedup 1.0000x reference)
"""Bass/Trainium2 kernel for BestMatchDistance.

ref: sim[b,q,s] = sum_d q[b,d,q]*s[b,d,s]; out[b] = mean_q max_s sim.

Sharding: batch dim B=64 split across 8 cores (8 batches/core), pure data
parallel. Inputs are cast to bf16 on the host (full-rate PE, half DMA).

Per (batch, 128-query tile): the [128, 2048] sim row = 4 bf16 matmuls
(K=64, N=512) K-packed 2-up onto PE row-groups 0-63 / 64-127 (query data is
duplicated to both partition halves, support is split), into two 2-bank
PSUM tiles PA/PB (double buffered = 8 banks).

PSUM can only be read by DVE and ACT, one PSUM operand per instruction, at
1 f32/cycle/lane — that dual port is the eviction floor. Per tile:
  - ACT copies PB[1024] -> SBUF bf16 bc (~0.9us @ 1.2 GHz).
  - DVE runs ONE custom fused op TT_MAX_REDUCE_ANT (registered below via
    the concourse custom-DVE extension API): elementwise max of PA (f32
    PSUM, the single allowed PSUM operand, 1024 cycles @ 0.96 GHz) against
    bc (via the second SBUF read port, free), max-accumulated into
    accB[:, tile] — the whole 2048-col row max in one instruction, with
    no reduction tree afterwards. The stock alternatives can't do this:
    tensor_tensor_reduce faults on TRN2 HW, tensor_tensor_scan runs at
    ~2.6 cyc/elem, and a copy+tree burns ~1.5x more DVE time.
Per batch: mean over queries = free-dim reduce_sum of accB + ones-vector
matmul over partitions, scaled by 1/NQ.
"""

import re

import numpy as np

B, D, NQ, NS = 64, 64, 2048, 2048
N_CORES = 8
BPC = B // N_CORES  # batches per core

PA_W = 1024  # DVE custom-op in0 (PSUM)
PB_W = 1024  # ACT copy -> bc, custom-op in1

_cache = {}


def _ref_tt_max_reduce(in0, in1, c0, c1, c2):
    P = in0.shape[0]
    a = in0.astype(np.float32).reshape(P, -1)
    b = np.asarray(in1, np.float32).reshape(P, -1)
    body = np.maximum(a, b)
    seed = np.asarray(c1, np.float32).reshape(-1, 1)
    acc = np.maximum(np.maximum.reduce(body, axis=-1, keepdims=True), seed)
    return body, acc


def _get_dve_op():
    """Register (once) and return the fused max(in0,in1)+max-accum DVE op,
    built with concourse's custom-DVE authoring API (dve_ops.OPS et al. are
    the documented extension points). The op's uop table ships inside the
    compiled NEFF; numerics validated against the reference on HW."""
    if "op" in _cache:
        return _cache["op"]
    from concourse import dve_ops
    from concourse.dve_spec import Spec, Src0, Src1, C1, maxx
    from concourse.dve_ops import DveOp
    from concourse.dve_table_gen import dve_ver_for, free_opcode_rows

    name = "TT_MAX_REDUCE_ANT"
    registered = next((o for o in dve_ops.OPS if o.name == name), None)
    if registered is not None:
        _cache["op"] = registered
        return registered

    spec = Spec(
        body=maxx(Src0, Src1),
        accum=maxx,
        accum_init=C1,
        reference=_ref_tt_max_reduce,
    )
    ver = dve_ver_for("TRN2")
    used_rows = set(dve_ops._SUB_OPCODE_FOR_NAME.values())
    row = next(r for r in free_opcode_rows("TRN2") if r not in used_rows)
    dve_ops._SUB_OPCODE_FOR_NAME[name] = row
    dve_ops.CUSTOM_DVE_SPECS[name] = spec
    probe = DveOp(name, spec, subdim=False, uops_sha={})
    try:
        probe.compile(ver)
        op = probe
    except ValueError as e:
        m = re.search(r"(v\d): ([0-9a-f]+) ", str(e))
        assert m, str(e)
        op = DveOp(name, spec, subdim=False, uops_sha={m.group(1): m.group(2)})
        op.compile(ver)
    dve_ops.OPS.append(op)
    _cache["op"] = op
    return op


def _emit_body(nc, mybir, q_d, s_d, o_d, ones, rall, pools, rep=0, parts=31):
    DO_MM = parts & 1
    DO_MR = parts & 2
    DO_CP = parts & 4
    f32 = mybir.dt.float32
    bf16 = mybir.dt.bfloat16
    X = mybir.AxisListType.X
    qp, sp, ppa, ppb, bcp, scp, rp, finp = pools
    dve_op = _get_dve_op()

    n_qt = NQ // 128  # 16 q-tiles per batch
    HNS = NS // 2  # support cols per PE row-group

    # Preload the ACT LUT table (~1.3us) concurrently with the first DMAs
    # instead of stalling the first PB copy on it.
    warm = finp.tile([1, 1], f32, tag="warm", name=f"warm{rep}")
    nc.scalar.copy(out=warm[:], in_=ones[0:1, :])

    for b in range(BPC):
        qt = qp.tile([128, NQ], bf16, tag="q", name=f"q{rep}_{b}")
        st = sp.tile([128, HNS], bf16, tag="s", name=f"s{rep}_{b}")
        if b == 0:
            # split batch 0's input DMAs so the first matmuls (which only
            # need the leading s/q columns) start sooner; the first PB
            # matmul needs st[0:64, 0:512] + qt[0:64, 0:128] -> those two
            # chunks are issued first
            nc.sync.dma_start(out=st[0:64, 0:512], in_=s_d[b][:, 0:512])
            nc.sync.dma_start(out=qt[0:64, 0:512], in_=q_d[b][:, 0:512])
            nc.sync.dma_start(out=st[64:128, 0:512], in_=s_d[b][:, HNS : HNS + 512])
            nc.sync.dma_start(out=qt[64:128, 0:512], in_=q_d[b][:, 0:512])
            nc.sync.dma_start(out=st[0:64, 512:HNS], in_=s_d[b][:, 512:HNS])
            nc.sync.dma_start(
                out=st[64:128, 512:HNS], in_=s_d[b][:, HNS + 512 : NS]
            )
            nc.sync.dma_start(out=qt[0:64, 512:NQ], in_=q_d[b][:, 512:NQ])
            nc.sync.dma_start(out=qt[64:128, 512:NQ], in_=q_d[b][:, 512:NQ])
        else:
            nc.sync.dma_start(out=qt[0:64, :], in_=q_d[b])
            nc.sync.dma_start(out=qt[64:128, :], in_=q_d[b])
            nc.sync.dma_start(out=st[0:64, :], in_=s_d[b][:, 0:HNS])
            nc.sync.dma_start(out=st[64:128, :], in_=s_d[b][:, HNS:NS])

        accB = rp.tile([128, n_qt], f32, tag="accB", name=f"accB{rep}_{b}")
        if not (DO_MR and DO_CP):
            nc.vector.memset(accB[:], 0.0)

        for i in range(n_qt):
            pa = ppa.tile([128, PA_W], f32, tag="PA", name=f"PA{rep}_{b}_{i}")
            pb = ppb.tile([128, PB_W], f32, tag="PB", name=f"PB{rep}_{b}_{i}")
            lhs0 = qt[0:64, i * 128 : (i + 1) * 128]
            lhs1 = qt[64:128, i * 128 : (i + 1) * 128]
            if DO_MM:
                # 4 N=512 matmuls, K-packed: grp0 covers s-cols [0,HNS),
                # grp1 covers [HNS,NS). The s-columns land permuted across
                # PSUM, which is irrelevant under a max-reduce. Each matmul
                # writes one 512-col chunk within one PSUM bank.
                dsts = [(pb, 0), (pb, 512), (pa, 0), (pa, 512)]
                for k4 in range(4):
                    grp = k4 % 2
                    sc = (k4 // 2) * 512
                    dst, off = dsts[k4]
                    if grp == 0:
                        nc.tensor.matmul(
                            dst[:, off : off + 512], lhsT=lhs0,
                            rhs=st[0:64, sc : sc + 512],
                            start=True, stop=True,
                        )
                    else:
                        nc.tensor.matmul(
                            dst[:, off : off + 512], lhsT=lhs1,
                            rhs=st[64:128, sc : sc + 512],
                            start=True, stop=True, tile_position=(64, 0),
                        )
            bc = bcp.tile([128, PB_W], bf16, tag="bc", name=f"bc{rep}_{b}_{i}")
            if DO_CP:
                nc.scalar.copy(out=bc[:], in_=pb[:])
            if DO_MR and DO_CP:
                scr = scp.tile(
                    [128, PA_W], bf16, tag="scr", name=f"sc{rep}_{b}_{i}"
                )
                nc.vector._custom_dve(
                    dve_op,
                    out=scr[:],
                    in0=pa[:],
                    in1=bc[:],
                    s1=-3.0e38,
                    accum_out=accB[:, i : i + 1],
                )

        nc.vector.reduce_sum(rall[:, b : b + 1], accB[:], axis=X)

    pf = ppa.tile([1, BPC], f32, tag="PA", name=f"pf{rep}")
    nc.tensor.matmul(pf[:], lhsT=ones[:], rhs=rall[:], start=True, stop=True)
    ob = finp.tile([1, BPC], f32, tag="ob", name=f"ob{rep}")
    nc.scalar.mul(ob[:], pf[:], 1.0 / NQ)
    nc.sync.dma_start(out=o_d[:], in_=ob[:])


def _build(loop_reps=None, parts=31):
    import concourse.bacc as bacc
    import concourse.mybir as mybir
    import concourse.tile as tile

    f32 = mybir.dt.float32
    bf16 = mybir.dt.bfloat16

    nc = bacc.Bacc("TRN2", target_bir_lowering=False, debug=False)
    q_d = nc.dram_tensor("q", [BPC, D, NQ], bf16, kind="ExternalInput").ap()
    s_d = nc.dram_tensor("s", [BPC, D, NS], bf16, kind="ExternalInput").ap()
    o_d = nc.dram_tensor("o", [1, BPC], f32, kind="ExternalOutput").ap()

    with tile.TileContext(nc) as tc:
        with (
            tc.tile_pool(name="scp", bufs=2) as scp,
            tc.tile_pool(name="qp", bufs=3) as qp,
            tc.tile_pool(name="sp", bufs=3) as sp,
            tc.tile_pool(name="ppa", bufs=2, space="PSUM") as ppa,
            tc.tile_pool(name="ppb", bufs=2, space="PSUM") as ppb,
            tc.tile_pool(name="rp", bufs=2) as rp,
            tc.tile_pool(name="fin", bufs=1) as finp,
            tc.tile_pool(name="bcp", bufs=2) as bcp,
        ):
            ones = finp.tile([128, 1], f32, tag="ones")
            nc.vector.memset(ones[:], 1.0)
            rall = finp.tile([128, BPC], f32, tag="rall")
            pools = (qp, sp, ppa, ppb, bcp, scp, rp, finp)

            if loop_reps is None:
                _emit_body(nc, mybir, q_d, s_d, o_d, ones, rall, pools, parts=parts)
            else:
                with tc.For_i(0, loop_reps, 1):
                    _emit_body(
                        nc, mybir, q_d, s_d, o_d, ones, rall, pools, parts=parts
                    )

    nc.compile()
    return nc


def _to_bf16(x):
    import ml_dtypes

    return np.ascontiguousarray(x, dtype=np.float32).astype(ml_dtypes.bfloat16)


def kernel(query_local, support_local):
    from concourse.bass_utils import run_bass_kernel_spmd

    if "nc" not in _cache:
        _cache["nc"] = _build()
    nc = _cache["nc"]

    q = _to_bf16(query_local).reshape(N_CORES, BPC, D, NQ)
    s = _to_bf16(support_local).reshape(N_CORES, BPC, D, NS)
    in_maps = [{"q": q[c], "s": s[c]} for c in range(N_CORES)]
    res = run_bass_kernel_spmd(nc, in_maps, list(range(N_CORES)))
    outs = [np.asarray(res.results[c]["o"]).reshape(BPC) for c in range(N_CORES)]
    return np.concatenate(outs, axis=0)


# revision 22
# speedup vs baseline: 1.2180x; 1.2180x over previous
"""Bass/Trainium2 kernel for BestMatchDistance.

ref: sim[b,q,s] = sum_d q[b,d,q]*s[b,d,s]; out[b] = mean_q max_s sim.

Sharding: batch dim B=64 split across 8 cores (8 batches/core), pure data
parallel. Inputs are cast to bf16 on the host (full-rate PE, half DMA).

Per (batch, 128-query tile): the [128, 2048] sim row = 4 bf16 matmuls
(K=64, N=512) K-packed 2-up onto PE row-groups 0-63 / 64-127 (query data is
duplicated to both partition halves, support is split), into two 2-bank
PSUM tiles PA/PB (double buffered = 8 banks).

PSUM can only be read by DVE and ACT, one PSUM operand per instruction, at
1 f32/cycle/lane — that dual port is the eviction floor. Per tile:
  - ACT copies PB[1024] -> SBUF bf16 bc (~0.9us @ 1.2 GHz).
  - DVE runs ONE custom fused op TT_MAX_REDUCE_ANT (registered below via
    the concourse custom-DVE extension API): elementwise max of PA (f32
    PSUM, the single allowed PSUM operand, 1024 cycles @ 0.96 GHz) against
    bc (via the second SBUF read port, free), max-accumulated into
    accB[:, tile] — the whole 2048-col row max in one instruction, with
    no reduction tree afterwards. The stock alternatives can't do this:
    tensor_tensor_reduce faults on TRN2 HW, tensor_tensor_scan runs at
    ~2.6 cyc/elem, and a copy+tree burns ~1.5x more DVE time.
Per batch: mean over queries = free-dim reduce_sum of accB + ones-vector
matmul over partitions, scaled by 1/NQ.
"""

import re

import numpy as np

B, D, NQ, NS = 64, 64, 2048, 2048
N_CORES = 8
BPC = B // N_CORES  # batches per core

PA_W = 1024  # DVE custom-op in0 (PSUM)
PB_W = 1024  # ACT copy -> bc, custom-op in1

_cache = {}


def _ref_tt_max_reduce(in0, in1, c0, c1, c2):
    P = in0.shape[0]
    a = in0.astype(np.float32).reshape(P, -1)
    b = np.asarray(in1, np.float32).reshape(P, -1)
    body = np.maximum(a, b)
    seed = np.asarray(c1, np.float32).reshape(-1, 1)
    acc = np.maximum(np.maximum.reduce(body, axis=-1, keepdims=True), seed)
    return body, acc


def _get_dve_op():
    """Register (once) and return the fused max(in0,in1)+max-accum DVE op,
    built with concourse's custom-DVE authoring API (dve_ops.OPS et al. are
    the documented extension points). The op's uop table ships inside the
    compiled NEFF; numerics validated against the reference on HW."""
    if "op" in _cache:
        return _cache["op"]
    from concourse import dve_ops
    from concourse.dve_spec import Spec, Src0, Src1, C1, maxx
    from concourse.dve_ops import DveOp
    from concourse.dve_table_gen import dve_ver_for, free_opcode_rows

    name = "TT_MAX_REDUCE_ANT"
    registered = next((o for o in dve_ops.OPS if o.name == name), None)
    if registered is not None:
        _cache["op"] = registered
        return registered

    spec = Spec(
        body=maxx(Src0, Src1),
        accum=maxx,
        accum_init=C1,
        reference=_ref_tt_max_reduce,
    )
    ver = dve_ver_for("TRN2")
    used_rows = set(dve_ops._SUB_OPCODE_FOR_NAME.values())
    row = next(r for r in free_opcode_rows("TRN2") if r not in used_rows)
    dve_ops._SUB_OPCODE_FOR_NAME[name] = row
    dve_ops.CUSTOM_DVE_SPECS[name] = spec
    probe = DveOp(name, spec, subdim=False, uops_sha={})
    try:
        probe.compile(ver)
        op = probe
    except ValueError as e:
        m = re.search(r"(v\d): ([0-9a-f]+) ", str(e))
        assert m, str(e)
        op = DveOp(name, spec, subdim=False, uops_sha={m.group(1): m.group(2)})
        op.compile(ver)
    dve_ops.OPS.append(op)
    _cache["op"] = op
    return op


def _emit_body(nc, mybir, q_d, s_d, o_d, ones, rall, pools, rep=0, parts=31):
    DO_MM = parts & 1
    DO_MR = parts & 2
    DO_CP = parts & 4
    f32 = mybir.dt.float32
    bf16 = mybir.dt.bfloat16
    X = mybir.AxisListType.X
    qp, sp, ppa, ppb, bcp, scp, rp, finp = pools
    dve_op = _get_dve_op()

    n_qt = NQ // 128  # 16 q-tiles per batch
    HNS = NS // 2  # support cols per PE row-group

    # Preload the ACT LUT table (~1.3us) concurrently with the first DMAs
    # instead of stalling the first PB copy on it.
    warm = finp.tile([1, 1], f32, tag="warm", name=f"warm{rep}")
    nc.scalar.copy(out=warm[:], in_=ones[0:1, :])

    for b in range(BPC):
        qt = qp.tile([128, NQ], bf16, tag="q", name=f"q{rep}_{b}")
        st = sp.tile([128, HNS], bf16, tag="s", name=f"s{rep}_{b}")
        if b == 0:
            # split batch 0's input DMAs so the first matmuls (which only
            # need the leading s/q columns) start sooner; the first PB
            # matmul needs st[0:64, 0:512] + qt[0:64, 0:128] -> those two
            # chunks are issued first
            nc.sync.dma_start(out=st[0:64, 0:512], in_=s_d[b][:, 0:512])
            nc.sync.dma_start(out=qt[0:64, 0:512], in_=q_d[b][:, 0:512])
            nc.sync.dma_start(out=st[64:128, 0:512], in_=s_d[b][:, HNS : HNS + 512])
            nc.sync.dma_start(out=qt[64:128, 0:512], in_=q_d[b][:, 0:512])
            nc.sync.dma_start(out=st[0:64, 512:HNS], in_=s_d[b][:, 512:HNS])
            nc.sync.dma_start(
                out=st[64:128, 512:HNS], in_=s_d[b][:, HNS + 512 : NS]
            )
            nc.sync.dma_start(out=qt[0:64, 512:NQ], in_=q_d[b][:, 512:NQ])
            nc.sync.dma_start(out=qt[64:128, 512:NQ], in_=q_d[b][:, 512:NQ])
        else:
            nc.sync.dma_start(out=qt[0:64, :], in_=q_d[b])
            nc.sync.dma_start(out=qt[64:128, :], in_=q_d[b])
            nc.sync.dma_start(out=st[0:64, :], in_=s_d[b][:, 0:HNS])
            nc.sync.dma_start(out=st[64:128, :], in_=s_d[b][:, HNS:NS])

        accB = rp.tile([128, n_qt], f32, tag="accB", name=f"accB{rep}_{b}")
        if not (DO_MR and DO_CP):
            nc.vector.memset(accB[:], 0.0)

        for i in range(n_qt):
            pa = ppa.tile([128, PA_W], f32, tag="PA", name=f"PA{rep}_{b}_{i}")
            pb = ppb.tile([128, PB_W], f32, tag="PB", name=f"PB{rep}_{b}_{i}")
            lhs0 = qt[0:64, i * 128 : (i + 1) * 128]
            lhs1 = qt[64:128, i * 128 : (i + 1) * 128]
            if DO_MM:
                # 4 N=512 matmuls, K-packed: grp0 covers s-cols [0,HNS),
                # grp1 covers [HNS,NS). The s-columns land permuted across
                # PSUM, which is irrelevant under a max-reduce. Each matmul
                # writes one 512-col chunk within one PSUM bank.
                dsts = [(pb, 0), (pb, 512), (pa, 0), (pa, 512)]
                for k4 in range(4):
                    grp = k4 % 2
                    sc = (k4 // 2) * 512
                    dst, off = dsts[k4]
                    if grp == 0:
                        nc.tensor.matmul(
                            dst[:, off : off + 512], lhsT=lhs0,
                            rhs=st[0:64, sc : sc + 512],
                            start=True, stop=True,
                        )
                    else:
                        nc.tensor.matmul(
                            dst[:, off : off + 512], lhsT=lhs1,
                            rhs=st[64:128, sc : sc + 512],
                            start=True, stop=True, tile_position=(64, 0),
                        )
            bc = bcp.tile([128, PB_W], bf16, tag="bc", name=f"bc{rep}_{b}_{i}")
            if DO_CP:
                nc.scalar.copy(out=bc[:], in_=pb[:])
            if DO_MR and DO_CP:
                scr = scp.tile(
                    [128, PA_W], bf16, tag="scr", name=f"sc{rep}_{b}_{i}"
                )
                nc.vector._custom_dve(
                    dve_op,
                    out=scr[:],
                    in0=pa[:],
                    in1=bc[:],
                    s1=-3.0e38,
                    accum_out=accB[:, i : i + 1],
                )

        nc.vector.reduce_sum(rall[:, b : b + 1], accB[:], axis=X)

    pf = ppa.tile([1, BPC], f32, tag="PA", name=f"pf{rep}")
    nc.tensor.matmul(pf[:], lhsT=ones[:], rhs=rall[:], start=True, stop=True)
    ob = finp.tile([1, BPC], f32, tag="ob", name=f"ob{rep}")
    nc.scalar.mul(ob[:], pf[:], 1.0 / NQ)
    nc.sync.dma_start(out=o_d[:], in_=ob[:])


def _build(loop_reps=None, parts=31):
    import concourse.bacc as bacc
    import concourse.mybir as mybir
    import concourse.tile as tile

    f32 = mybir.dt.float32
    bf16 = mybir.dt.bfloat16

    nc = bacc.Bacc("TRN2", target_bir_lowering=False, debug=False)
    q_d = nc.dram_tensor("q", [BPC, D, NQ], bf16, kind="ExternalInput").ap()
    s_d = nc.dram_tensor("s", [BPC, D, NS], bf16, kind="ExternalInput").ap()
    o_d = nc.dram_tensor("o", [1, BPC], f32, kind="ExternalOutput").ap()

    with tile.TileContext(nc) as tc:
        with (
            tc.tile_pool(name="scp", bufs=3) as scp,
            tc.tile_pool(name="qp", bufs=3) as qp,
            tc.tile_pool(name="sp", bufs=3) as sp,
            tc.tile_pool(name="ppa", bufs=2, space="PSUM") as ppa,
            tc.tile_pool(name="ppb", bufs=2, space="PSUM") as ppb,
            tc.tile_pool(name="rp", bufs=2) as rp,
            tc.tile_pool(name="fin", bufs=1) as finp,
            tc.tile_pool(name="bcp", bufs=3) as bcp,
        ):
            ones = finp.tile([128, 1], f32, tag="ones")
            nc.vector.memset(ones[:], 1.0)
            rall = finp.tile([128, BPC], f32, tag="rall")
            pools = (qp, sp, ppa, ppb, bcp, scp, rp, finp)

            if loop_reps is None:
                _emit_body(nc, mybir, q_d, s_d, o_d, ones, rall, pools, parts=parts)
            else:
                with tc.For_i(0, loop_reps, 1):
                    _emit_body(
                        nc, mybir, q_d, s_d, o_d, ones, rall, pools, parts=parts
                    )

    nc.compile()
    return nc


def _to_bf16(x):
    import ml_dtypes

    return np.ascontiguousarray(x, dtype=np.float32).astype(ml_dtypes.bfloat16)


def kernel(query_local, support_local):
    from concourse.bass_utils import run_bass_kernel_spmd

    if "nc" not in _cache:
        _cache["nc"] = _build()
    nc = _cache["nc"]

    q = _to_bf16(query_local).reshape(N_CORES, BPC, D, NQ)
    s = _to_bf16(support_local).reshape(N_CORES, BPC, D, NS)
    in_maps = [{"q": q[c], "s": s[c]} for c in range(N_CORES)]
    res = run_bass_kernel_spmd(nc, in_maps, list(range(N_CORES)))
    outs = [np.asarray(res.results[c]["o"]).reshape(BPC) for c in range(N_CORES)]
    return np.concatenate(outs, axis=0)


# revision 23
# speedup vs baseline: 1.2316x; 1.0112x over previous
"""Bass/Trainium2 kernel for BestMatchDistance.

ref: sim[b,q,s] = sum_d q[b,d,q]*s[b,d,s]; out[b] = mean_q max_s sim.

Sharding: batch dim B=64 split across 8 cores (8 batches/core), pure data
parallel. Inputs are cast to bf16 on the host (full-rate PE, half DMA).

Per (batch, 128-query tile): the [128, 2048] sim row = 4 bf16 matmuls
(K=64, N=512) K-packed 2-up onto PE row-groups 0-63 / 64-127 (query data is
duplicated to both partition halves, support is split), into two 2-bank
PSUM tiles PA/PB (double buffered = 8 banks).

PSUM can only be read by DVE and ACT, one PSUM operand per instruction, at
1 f32/cycle/lane — that dual port is the eviction floor. Per tile:
  - ACT copies PB[1024] -> SBUF bf16 bc (~0.9us @ 1.2 GHz).
  - DVE runs ONE custom fused op TT_MAX_REDUCE_ANT (registered below via
    the concourse custom-DVE extension API): elementwise max of PA (f32
    PSUM, the single allowed PSUM operand, 1024 cycles @ 0.96 GHz) against
    bc (via the second SBUF read port, free), max-accumulated into
    accB[:, tile] — the whole 2048-col row max in one instruction, with
    no reduction tree afterwards. The stock alternatives can't do this:
    tensor_tensor_reduce faults on TRN2 HW, tensor_tensor_scan runs at
    ~2.6 cyc/elem, and a copy+tree burns ~1.5x more DVE time.
Per batch: mean over queries = free-dim reduce_sum of accB + ones-vector
matmul over partitions, scaled by 1/NQ.
"""

import re

import numpy as np

B, D, NQ, NS = 64, 64, 2048, 2048
N_CORES = 8
BPC = B // N_CORES  # batches per core

PA_W = 1024  # DVE custom-op in0 (PSUM)
PB_W = 1024  # ACT copy -> bc, custom-op in1

_cache = {}


def _ref_tt_max_reduce(in0, in1, c0, c1, c2):
    P = in0.shape[0]
    a = in0.astype(np.float32).reshape(P, -1)
    b = np.asarray(in1, np.float32).reshape(P, -1)
    body = np.maximum(a, b)
    seed = np.asarray(c1, np.float32).reshape(-1, 1)
    acc = np.maximum(np.maximum.reduce(body, axis=-1, keepdims=True), seed)
    return body, acc


def _get_dve_op():
    """Register (once) and return the fused max(in0,in1)+max-accum DVE op,
    built with concourse's custom-DVE authoring API (dve_ops.OPS et al. are
    the documented extension points). The op's uop table ships inside the
    compiled NEFF; numerics validated against the reference on HW."""
    if "op" in _cache:
        return _cache["op"]
    from concourse import dve_ops
    from concourse.dve_spec import Spec, Src0, Src1, C1, maxx
    from concourse.dve_ops import DveOp
    from concourse.dve_table_gen import dve_ver_for, free_opcode_rows

    name = "TT_MAX_REDUCE_ANT"
    registered = next((o for o in dve_ops.OPS if o.name == name), None)
    if registered is not None:
        _cache["op"] = registered
        return registered

    spec = Spec(
        body=maxx(Src0, Src1),
        accum=maxx,
        accum_init=C1,
        reference=_ref_tt_max_reduce,
    )
    ver = dve_ver_for("TRN2")
    used_rows = set(dve_ops._SUB_OPCODE_FOR_NAME.values())
    row = next(r for r in free_opcode_rows("TRN2") if r not in used_rows)
    dve_ops._SUB_OPCODE_FOR_NAME[name] = row
    dve_ops.CUSTOM_DVE_SPECS[name] = spec
    probe = DveOp(name, spec, subdim=False, uops_sha={})
    try:
        probe.compile(ver)
        op = probe
    except ValueError as e:
        m = re.search(r"(v\d): ([0-9a-f]+) ", str(e))
        assert m, str(e)
        op = DveOp(name, spec, subdim=False, uops_sha={m.group(1): m.group(2)})
        op.compile(ver)
    dve_ops.OPS.append(op)
    _cache["op"] = op
    return op


def _emit_body(nc, mybir, q_d, s_d, o_d, ones, rall, pools, rep=0, parts=31):
    DO_MM = parts & 1
    DO_MR = parts & 2
    DO_CP = parts & 4
    f32 = mybir.dt.float32
    bf16 = mybir.dt.bfloat16
    X = mybir.AxisListType.X
    qp, sp, ppa, ppb, bcp, scp, rp, finp = pools
    dve_op = _get_dve_op()

    n_qt = NQ // 128  # 16 q-tiles per batch
    HNS = NS // 2  # support cols per PE row-group

    # Preload the ACT LUT table (~1.3us) concurrently with the first DMAs
    # instead of stalling the first PB copy on it.
    warm = finp.tile([1, 1], f32, tag="warm", name=f"warm{rep}")
    nc.scalar.copy(out=warm[:], in_=ones[0:1, :])

    for b in range(BPC):
        qt = qp.tile([128, NQ], bf16, tag="q", name=f"q{rep}_{b}")
        st = sp.tile([128, HNS], bf16, tag="s", name=f"s{rep}_{b}")
        if b == 0:
            # split batch 0's input DMAs so the first matmuls (which only
            # need the leading s/q columns) start sooner; the first PB
            # matmul needs st[0:64, 0:512] + qt[0:64, 0:128] -> those two
            # chunks are issued first
            nc.sync.dma_start(out=st[0:64, 0:512], in_=s_d[b][:, 0:512])
            nc.sync.dma_start(out=qt[0:64, 0:512], in_=q_d[b][:, 0:512])
            nc.sync.dma_start(out=st[64:128, 0:512], in_=s_d[b][:, HNS : HNS + 512])
            nc.sync.dma_start(out=qt[64:128, 0:512], in_=q_d[b][:, 0:512])
            nc.sync.dma_start(out=st[0:64, 512:HNS], in_=s_d[b][:, 512:HNS])
            nc.sync.dma_start(
                out=st[64:128, 512:HNS], in_=s_d[b][:, HNS + 512 : NS]
            )
            nc.sync.dma_start(out=qt[0:64, 512:NQ], in_=q_d[b][:, 512:NQ])
            nc.sync.dma_start(out=qt[64:128, 512:NQ], in_=q_d[b][:, 512:NQ])
        else:
            nc.sync.dma_start(out=qt[0:64, :], in_=q_d[b])
            nc.sync.dma_start(out=qt[64:128, :], in_=q_d[b])
            nc.sync.dma_start(out=st[0:64, :], in_=s_d[b][:, 0:HNS])
            nc.sync.dma_start(out=st[64:128, :], in_=s_d[b][:, HNS:NS])

        accB = rp.tile([128, n_qt], f32, tag="accB", name=f"accB{rep}_{b}")
        if not (DO_MR and DO_CP):
            nc.vector.memset(accB[:], 0.0)

        for i in range(n_qt):
            pa = ppa.tile([128, PA_W], f32, tag="PA", name=f"PA{rep}_{b}_{i}")
            pb = ppb.tile([128, PB_W], f32, tag="PB", name=f"PB{rep}_{b}_{i}")
            lhs0 = qt[0:64, i * 128 : (i + 1) * 128]
            lhs1 = qt[64:128, i * 128 : (i + 1) * 128]
            if DO_MM:
                # 4 N=512 matmuls, K-packed: grp0 covers s-cols [0,HNS),
                # grp1 covers [HNS,NS). The s-columns land permuted across
                # PSUM, which is irrelevant under a max-reduce. Each matmul
                # writes one 512-col chunk within one PSUM bank.
                dsts = [(pb, 0), (pb, 512), (pa, 0), (pa, 512)]
                for k4 in range(4):
                    grp = k4 % 2
                    sc = (k4 // 2) * 512
                    dst, off = dsts[k4]
                    if grp == 0:
                        nc.tensor.matmul(
                            dst[:, off : off + 512], lhsT=lhs0,
                            rhs=st[0:64, sc : sc + 512],
                            start=True, stop=True,
                        )
                    else:
                        nc.tensor.matmul(
                            dst[:, off : off + 512], lhsT=lhs1,
                            rhs=st[64:128, sc : sc + 512],
                            start=True, stop=True, tile_position=(64, 0),
                        )
            bc = bcp.tile([128, PB_W], bf16, tag="bc", name=f"bc{rep}_{b}_{i}")
            if DO_CP:
                nc.scalar.copy(out=bc[:], in_=pb[:])
            if DO_MR and DO_CP:
                scr = scp.tile(
                    [128, PA_W], bf16, tag="scr", name=f"sc{rep}_{b}_{i}"
                )
                nc.vector._custom_dve(
                    dve_op,
                    out=scr[:],
                    in0=pa[:],
                    in1=bc[:],
                    s1=-3.0e38,
                    accum_out=accB[:, i : i + 1],
                )

        nc.vector.reduce_sum(rall[:, b : b + 1], accB[:], axis=X)

    pf = ppa.tile([1, BPC], f32, tag="PA", name=f"pf{rep}")
    nc.tensor.matmul(pf[:], lhsT=ones[:], rhs=rall[:], start=True, stop=True)
    ob = finp.tile([1, BPC], f32, tag="ob", name=f"ob{rep}")
    nc.scalar.mul(ob[:], pf[:], 1.0 / NQ)
    nc.sync.dma_start(out=o_d[:], in_=ob[:])


def _build(loop_reps=None, parts=31):
    import concourse.bacc as bacc
    import concourse.mybir as mybir
    import concourse.tile as tile

    f32 = mybir.dt.float32
    bf16 = mybir.dt.bfloat16

    nc = bacc.Bacc("TRN2", target_bir_lowering=False, debug=False)
    q_d = nc.dram_tensor("q", [BPC, D, NQ], bf16, kind="ExternalInput").ap()
    s_d = nc.dram_tensor("s", [BPC, D, NS], bf16, kind="ExternalInput").ap()
    o_d = nc.dram_tensor("o", [1, BPC], f32, kind="ExternalOutput").ap()

    with tile.TileContext(nc) as tc:
        with (
            tc.tile_pool(name="scp", bufs=3) as scp,
            tc.tile_pool(name="qp", bufs=2) as qp,
            tc.tile_pool(name="sp", bufs=2) as sp,
            tc.tile_pool(name="ppa", bufs=2, space="PSUM") as ppa,
            tc.tile_pool(name="ppb", bufs=2, space="PSUM") as ppb,
            tc.tile_pool(name="rp", bufs=2) as rp,
            tc.tile_pool(name="fin", bufs=1) as finp,
            tc.tile_pool(name="bcp", bufs=3) as bcp,
        ):
            ones = finp.tile([128, 1], f32, tag="ones")
            nc.vector.memset(ones[:], 1.0)
            rall = finp.tile([128, BPC], f32, tag="rall")
            pools = (qp, sp, ppa, ppb, bcp, scp, rp, finp)

            if loop_reps is None:
                _emit_body(nc, mybir, q_d, s_d, o_d, ones, rall, pools, parts=parts)
            else:
                with tc.For_i(0, loop_reps, 1):
                    _emit_body(
                        nc, mybir, q_d, s_d, o_d, ones, rall, pools, parts=parts
                    )

    nc.compile()
    return nc


def _to_bf16(x):
    import ml_dtypes

    return np.ascontiguousarray(x, dtype=np.float32).astype(ml_dtypes.bfloat16)


def kernel(query_local, support_local):
    from concourse.bass_utils import run_bass_kernel_spmd

    if "nc" not in _cache:
        _cache["nc"] = _build()
    nc = _cache["nc"]

    q = _to_bf16(query_local).reshape(N_CORES, BPC, D, NQ)
    s = _to_bf16(support_local).reshape(N_CORES, BPC, D, NS)
    in_maps = [{"q": q[c], "s": s[c]} for c in range(N_CORES)]
    res = run_bass_kernel_spmd(nc, in_maps, list(range(N_CORES)))
    outs = [np.asarray(res.results[c]["o"]).reshape(BPC) for c in range(N_CORES)]
    return np.concatenate(outs, axis=0)
